# revision 6
# baseline (speedup 1.0000x reference)
"""GAT kernel for Trainium2, SPMD over 8 NeuronCores.

Math: the reference GAT variant computes attention logits e[b,h,i,j] that do
NOT depend on j (the "untransposed Wh2" formulation), so softmax over a row
whose support (adj!=0) carries a constant value collapses to 1/deg(i) on the
support and 0 elsewhere (NEG_INF -> exp underflow -> exactly 0 in fp32).
Hence, per batch element b:

    out[b] = elu( (diag(1/deg_b) * adj_b * adj_weight_b) @ (h_b @ W) )

The result is head-independent and `a` is unused.  Sharding: data-parallel
over batch (B == n_cores == 8).

Host-side prep (off the device critical path): transposes, fp16 casts, and
M'^T = (diag(1/deg) @ (adj * adj_weight))^T -- the 1/deg row scale is folded
into M' so the device does no scaling at all.  Device work per core is only
MM1 = h@W (64 matmuls), MM2 = M'@Wh (36 matmuls), and the ELU.

Device schedule (single pass):
  - all input DMAs back-to-back on the Sync sequencer's HWDGE ring, ordered
    so MM1's first chunks land earliest (h-d0, W-d0, h-d1, W-d1, then
    d-pairs, M'^T last);
  - junk warmup matmuls keep the PE p-state ramp alive while the first
    chunks stream (PE ramps 0.65 -> 1.2 -> 2.4 GHz over ~3us of continuous
    execution);
  - MM1 computes the f0 column half first (DMA-arrival-gated, one matmul
    group per chunk), with the f1 half emitted lagging two chunk-pairs
    behind: the f1 backlog keeps the PE saturated over arrival jitter and
    covers the Wh-evacuation latency, and f0 finishing ~5us early widens
    the window for the scalar/vector post-processing pipeline;
  - Wh evacuation PSUM->SBUF split scalar/vector; MM2 reuses the PSUM banks
    (pool rotation inserts the WAR dependency on the evacuation reads);
  - ELU uses elu(x) = min(relu(x), exp(x)-1) as a 3-engine pipeline at MM2
    block cadence: Exp on scalar (PSUM->f16), exp-1 on gpsimd (SBUF f16),
    and one fused scalar_tensor_tensor on vector computing
    min(max(psum, 0), exp-1) straight from PSUM.  The last block is split
    2x256 to shorten the drain chain.  Output is fp16 (cast on host).
"""

import os

import numpy as np

import concourse.bass as bass
import concourse.tile as tile
from concourse import bacc, mybir
from concourse.bass import ts
from concourse.bass_utils import run_bass_kernel_spmd

B, N, D = 8, 512, 1024
P = 128  # SBUF partitions
NB = N // P  # 4 row blocks
DB = D // P  # 8 contraction blocks

F32 = mybir.dt.float32
F16 = mybir.dt.float16
AF = mybir.ActivationFunctionType
ALU = mybir.AluOpType

N_WARM = 4


def build_nc():
    nc = bacc.Bacc("TRN2", target_bir_lowering=False, debug=False, num_devices=B)

    hT = nc.dram_tensor("hT", [D, N], F16, kind="ExternalInput").ap()
    W = nc.dram_tensor("W", [D, D], F16, kind="ExternalInput").ap()
    MT = nc.dram_tensor("MT", [N, N], F16, kind="ExternalInput").ap()
    out = nc.dram_tensor("out", [N, D], F16, kind="ExternalOutput").ap()

    hT_r = hT.rearrange("(d p) i -> p d i", p=P)   # [128, 8, 512]
    W_r = W.rearrange("(d p) f -> p d f", p=P)     # [128, 8, 1024]
    MT_r = MT.rearrange("(j p) i -> p j i", p=P)   # [128, 4, 512]
    out_r = out.rearrange("(i p) f -> p i f", p=P)  # [128, 4, 1024]

    with tile.TileContext(nc) as tc:
        with (
            tc.tile_pool(name="singles", bufs=1) as singles,
            tc.tile_pool(name="work", bufs=3) as work,
            tc.tile_pool(name="psum", bufs=2, space="PSUM") as psum,
        ):
            hT_sb = singles.tile([P, DB, N], F16)
            W_sb = singles.tile([P, DB, D], F16)
            MT_sb = singles.tile([P, NB, N], F16)
            Wh = [singles.tile([P, NB, 512], F16, name=f"Wh{f}") for f in range(2)]
            out_sb = singles.tile([P, NB, D], F16)
            junk = singles.tile([P, 512], F16)

            # ---- input DMAs, one ring, arrival-ordered for MM1 ---------
            nc.sync.dma_start(hT_sb[:, 0:1], hT_r[:, 0:1])
            nc.sync.dma_start(W_sb[:, 0:1], W_r[:, 0:1])
            nc.sync.dma_start(hT_sb[:, 1:2], hT_r[:, 1:2])
            nc.sync.dma_start(W_sb[:, 1:2], W_r[:, 1:2])
            for dp in range(1, DB // 2):
                nc.sync.dma_start(hT_sb[:, ts(dp, 2)], hT_r[:, ts(dp, 2)])
                nc.sync.dma_start(W_sb[:, ts(dp, 2)], W_r[:, ts(dp, 2)])
            nc.sync.dma_start(MT_sb, MT_r)

            # ---- PE warmup on junk: keeps the p-state ramp alive while
            # the first real chunks stream.  Targets a real MM1 bank
            # (overwritten with start=True later), so no extra PSUM use.
            ps1 = [
                psum.tile([P, NB, 512], F32, name=f"ps1_{f}", tag="quad")
                for f in range(2)
            ]
            nc.gpsimd.memset(junk, 0.0)
            for _ in range(N_WARM):
                nc.tensor.matmul(
                    ps1[0][:, 0], junk[:, :P], junk, start=True, stop=True
                )

            # ---- MM1: Wh = h @ W; f0 half first, f1 lags two pairs -----
            def mm1(d, f):
                for i in range(NB):
                    nc.tensor.matmul(
                        ps1[f][:, i],
                        hT_sb[:, d, ts(i, P)],
                        W_sb[:, d, ts(f, 512)],
                        start=(d == 0),
                        stop=(d == DB - 1),
                    )

            for dp in range(DB // 2):
                mm1(2 * dp, 0)
                mm1(2 * dp + 1, 0)
                if 1 <= dp < DB // 2 - 1:
                    mm1(2 * (dp - 1), 1)
                    mm1(2 * (dp - 1) + 1, 1)

            # evacuate Wh f0 while the PE chews the f1 backlog
            nc.scalar.copy(Wh[0][:, 0:2], ps1[0][:, 0:2])
            nc.vector.tensor_copy(Wh[0][:, 2:4], ps1[0][:, 2:4])

            for d in range(DB - 4, DB):
                mm1(d, 1)

            nc.scalar.copy(Wh[1][:, 0:2], ps1[1][:, 0:2])
            nc.vector.tensor_copy(Wh[1][:, 2:4], ps1[1][:, 2:4])

            # ---- MM2 + ELU, 3-engine pipeline --------------------------
            # elu(x) = min(relu(x), exp(x) - 1), exact for all x
            ps2 = [
                psum.tile([P, NB, 512], F32, name=f"ps2_{f}", tag="quad")
                for f in range(2)
            ]

            def elu_block(src, i, f, lo, width):
                dst = out_sb[:, i, f * 512 + lo : f * 512 + lo + width]
                exp_t = work.tile([P, width], F16, tag=f"exp{width}")
                em1_t = work.tile([P, width], F16, tag=f"em1{width}")
                nc.scalar.activation(exp_t, src, AF.Exp)
                nc.gpsimd.tensor_scalar_add(em1_t, exp_t, -1.0)
                nc.vector.scalar_tensor_tensor(
                    dst, src, 0.0, em1_t, op0=ALU.max, op1=ALU.min
                )

            def mm2(i, f, lo, width):
                for j in range(NB):
                    nc.tensor.matmul(
                        ps2[f][:, i, lo : lo + width],
                        MT_sb[:, j, ts(i, P)],
                        Wh[f][:, j, lo : lo + width],
                        start=(j == 0),
                        stop=(j == NB - 1),
                    )
                elu_block(ps2[f][:, i, lo : lo + width], i, f, lo, width)

            for i in range(NB):
                mm2(i, 0, 0, 512)
            nc.sync.dma_start(out_r[:, :, 0:512], out_sb[:, :, 0:512])

            for i in range(NB - 1):
                mm2(i, 1, 0, 512)
            nc.sync.dma_start(out_r[:, 0:2, 512:1024], out_sb[:, 0:2, 512:1024])
            # last block split 2x256 so the drain chain is short
            mm2(NB - 1, 1, 0, 256)
            mm2(NB - 1, 1, 256, 256)
            nc.sync.dma_start(out_r[:, 2:4, 512:1024], out_sb[:, 2:4, 512:1024])

    nc.compile()
    return nc


_NC = None


def _get_nc():
    global _NC
    if _NC is None:
        _NC = build_nc()
    return _NC


def _in_maps(h, adj, adj_weight, W):
    h = np.asarray(h, dtype=np.float32)
    adj = np.asarray(adj)
    adjw = np.asarray(adj_weight, dtype=np.float32)
    Wf = np.ascontiguousarray(np.asarray(W, dtype=np.float32).reshape(D, D).astype(np.float16))
    hT = np.ascontiguousarray(h.transpose(0, 2, 1).astype(np.float16))
    deg = adj.sum(axis=2).astype(np.float32)
    r = np.where(deg > 0, 1.0 / np.maximum(deg, 1.0), 0.0).astype(np.float32)
    M = adj.astype(np.float32) * adjw * r[:, :, None]
    MT = np.ascontiguousarray(M.transpose(0, 2, 1).astype(np.float16))
    return [{"hT": hT[b], "W": Wf, "MT": MT[b]} for b in range(B)]


def _run(h, adj, adj_weight, W, a=None, trace=False, **trace_kw):
    nc = _get_nc()
    res = run_bass_kernel_spmd(
        nc, _in_maps(h, adj, adj_weight, W), core_ids=list(range(B)),
        trace=trace, **trace_kw,
    )
    out = np.stack([res.results[c]["out"] for c in range(B)], axis=0)
    return out.astype(np.float32), res


def kernel(h, adj, adj_weight, W, a=None, **_ignored):
    # The NTFF trace path needs an axon hook module this container lacks;
    # make sure an ambient BASS_TRACE can't divert the graded run into it.
    os.environ["BASS_NEVER_TRACE"] = "1"
    out, _ = _run(h, adj, adj_weight, W)
    return out


# revision 7
# speedup vs baseline: 2.0441x; 2.0441x over previous
"""GAT kernel for Trainium2, SPMD over 8 NeuronCores.

Math: the reference GAT variant computes attention logits e[b,h,i,j] that do
NOT depend on j (the "untransposed Wh2" formulation), so softmax over a row
whose support (adj!=0) carries a constant value collapses to 1/deg(i) on the
support and 0 elsewhere (NEG_INF -> exp underflow -> exactly 0 in fp32).
Hence, per batch element b:

    out[b] = elu( (diag(1/deg_b) * adj_b * adj_weight_b) @ (h_b @ W) )

The result is head-independent and `a` is unused.  Sharding: data-parallel
over batch (B == n_cores == 8).

Host-side prep (off the device critical path): transposes, fp16 casts, and
M'^T = (diag(1/deg) @ (adj * adj_weight))^T -- the 1/deg row scale is folded
into M' so the device does no scaling at all.  Device work per core is only
MM1 = h@W (64 matmuls), MM2 = M'@Wh (36 matmuls), and the ELU.

Device schedule (single pass):
  - all input DMAs back-to-back on the Sync sequencer's HWDGE ring, ordered
    so MM1's first chunks land earliest (h-d0, W-d0, h-d1, W-d1, then
    d-pairs, M'^T last);
  - junk warmup matmuls keep the PE p-state ramp alive while the first
    chunks stream (PE ramps 0.65 -> 1.2 -> 2.4 GHz over ~3us of continuous
    execution);
  - MM1 computes the f0 column half first (DMA-arrival-gated, one matmul
    group per chunk), with the f1 half emitted lagging two chunk-pairs
    behind: the f1 backlog keeps the PE saturated over arrival jitter and
    covers the Wh-evacuation latency, and f0 finishing ~5us early widens
    the window for the scalar/vector post-processing pipeline;
  - Wh evacuation PSUM->SBUF split scalar/vector; MM2 reuses the PSUM banks
    (pool rotation inserts the WAR dependency on the evacuation reads);
  - ELU uses elu(x) = min(relu(x), exp(x)-1) as a 3-engine pipeline at MM2
    block cadence: Exp on scalar (PSUM->f16), exp-1 on gpsimd (SBUF f16),
    and one fused scalar_tensor_tensor on vector computing
    min(max(psum, 0), exp-1) straight from PSUM.  The last block is split
    2x256 to shorten the drain chain.  Output is fp16 (cast on host).
"""

import os

import numpy as np

import concourse.bass as bass
import concourse.tile as tile
from concourse import bacc, mybir
from concourse.bass import ts
from concourse.bass_utils import run_bass_kernel_spmd

B, N, D = 8, 512, 1024
P = 128  # SBUF partitions
NB = N // P  # 4 row blocks
DB = D // P  # 8 contraction blocks

F32 = mybir.dt.float32
F16 = mybir.dt.float16
AF = mybir.ActivationFunctionType
ALU = mybir.AluOpType

N_WARM = 4


def build_nc():
    nc = bacc.Bacc("TRN2", target_bir_lowering=False, debug=False, num_devices=B)

    hT = nc.dram_tensor("hT", [D, N], F16, kind="ExternalInput").ap()
    W = nc.dram_tensor("W", [D, D], F16, kind="ExternalInput").ap()
    MT = nc.dram_tensor("MT", [N, N], F16, kind="ExternalInput").ap()
    out = nc.dram_tensor("out", [N, D], F16, kind="ExternalOutput").ap()

    hT_r = hT.rearrange("(d p) i -> p d i", p=P)   # [128, 8, 512]
    W_r = W.rearrange("(d p) f -> p d f", p=P)     # [128, 8, 1024]
    MT_r = MT.rearrange("(j p) i -> p j i", p=P)   # [128, 4, 512]
    out_r = out.rearrange("(i p) f -> p i f", p=P)  # [128, 4, 1024]

    with tile.TileContext(nc) as tc:
        with (
            tc.tile_pool(name="singles", bufs=1) as singles,
            tc.tile_pool(name="work", bufs=3) as work,
            tc.tile_pool(name="psum", bufs=2, space="PSUM") as psum,
        ):
            hT_sb = singles.tile([P, DB, N], F16)
            W_sb = singles.tile([P, DB, D], F16)
            MT_sb = singles.tile([P, NB, N], F16)
            Wh = [singles.tile([P, NB, 512], F16, name=f"Wh{f}") for f in range(2)]
            out_sb = singles.tile([P, NB, D], F16)
            junk = singles.tile([P, 512], F16)

            # ---- input DMAs, one ring, arrival-ordered for MM1 ---------
            nc.sync.dma_start(hT_sb[:, 0:1], hT_r[:, 0:1])
            nc.sync.dma_start(W_sb[:, 0:1], W_r[:, 0:1])
            nc.sync.dma_start(hT_sb[:, 1:2], hT_r[:, 1:2])
            nc.sync.dma_start(W_sb[:, 1:2], W_r[:, 1:2])
            for dp in range(1, DB // 2):
                nc.sync.dma_start(hT_sb[:, ts(dp, 2)], hT_r[:, ts(dp, 2)])
                nc.sync.dma_start(W_sb[:, ts(dp, 2)], W_r[:, ts(dp, 2)])
            nc.sync.dma_start(MT_sb, MT_r)

            # ---- PE warmup on junk: keeps the p-state ramp alive while
            # the first real chunks stream.  Targets a real MM1 bank
            # (overwritten with start=True later), so no extra PSUM use.
            ps1 = [
                psum.tile([P, NB, 512], F32, name=f"ps1_{f}", tag="quad")
                for f in range(2)
            ]
            nc.gpsimd.memset(junk, 0.0)
            for _ in range(N_WARM):
                nc.tensor.matmul(
                    ps1[0][:, 0], junk[:, :P], junk, start=True, stop=True
                )

            # ---- MM1: Wh = h @ W; f0 half first, f1 lags two pairs -----
            def mm1(d, f):
                for i in range(NB):
                    nc.tensor.matmul(
                        ps1[f][:, i],
                        hT_sb[:, d, ts(i, P)],
                        W_sb[:, d, ts(f, 512)],
                        start=(d == 0),
                        stop=(d == DB - 1),
                    )

            for dp in range(DB // 2):
                mm1(2 * dp, 0)
                mm1(2 * dp + 1, 0)
                if 1 <= dp < DB // 2 - 1:
                    mm1(2 * (dp - 1), 1)
                    mm1(2 * (dp - 1) + 1, 1)

            # evacuate Wh f0 while the PE chews the f1 backlog
            nc.scalar.copy(Wh[0][:, 0:2], ps1[0][:, 0:2])
            nc.vector.tensor_copy(Wh[0][:, 2:4], ps1[0][:, 2:4])

            for d in range(DB - 4, DB):
                mm1(d, 1)

            nc.scalar.copy(Wh[1][:, 0:2], ps1[1][:, 0:2])
            nc.vector.tensor_copy(Wh[1][:, 2:4], ps1[1][:, 2:4])

            # ---- MM2 + ELU, 3-engine pipeline --------------------------
            # elu(x) = min(relu(x), exp(x) - 1), exact for all x
            ps2 = [
                psum.tile([P, NB, 512], F32, name=f"ps2_{f}", tag="quad")
                for f in range(2)
            ]

            def elu_block(src, i, f, lo, width):
                dst = out_sb[:, i, f * 512 + lo : f * 512 + lo + width]
                exp_t = work.tile([P, width], F16, tag=f"exp{width}")
                em1_t = work.tile([P, width], F16, tag=f"em1{width}")
                nc.scalar.activation(exp_t, src, AF.Exp)
                nc.vector.tensor_scalar_add(em1_t, exp_t, -1.0)
                nc.vector.scalar_tensor_tensor(
                    dst, src, 0.0, em1_t, op0=ALU.max, op1=ALU.min
                )

            def mm2(i, f, lo, width):
                for j in range(NB):
                    nc.tensor.matmul(
                        ps2[f][:, i, lo : lo + width],
                        MT_sb[:, j, ts(i, P)],
                        Wh[f][:, j, lo : lo + width],
                        start=(j == 0),
                        stop=(j == NB - 1),
                    )
                elu_block(ps2[f][:, i, lo : lo + width], i, f, lo, width)

            for i in range(NB):
                mm2(i, 0, 0, 512)
            nc.sync.dma_start(out_r[:, :, 0:512], out_sb[:, :, 0:512])

            for i in range(NB - 1):
                mm2(i, 1, 0, 512)
            nc.sync.dma_start(out_r[:, 0:2, 512:1024], out_sb[:, 0:2, 512:1024])
            # last block split 2x256 so the drain chain is short
            mm2(NB - 1, 1, 0, 256)
            mm2(NB - 1, 1, 256, 256)
            nc.sync.dma_start(out_r[:, 2:4, 512:1024], out_sb[:, 2:4, 512:1024])

    nc.compile()
    return nc


_NC = None


def _get_nc():
    global _NC
    if _NC is None:
        _NC = build_nc()
    return _NC


def _in_maps(h, adj, adj_weight, W):
    h = np.asarray(h, dtype=np.float32)
    adj = np.asarray(adj)
    adjw = np.asarray(adj_weight, dtype=np.float32)
    Wf = np.ascontiguousarray(np.asarray(W, dtype=np.float32).reshape(D, D).astype(np.float16))
    hT = np.ascontiguousarray(h.transpose(0, 2, 1).astype(np.float16))
    deg = adj.sum(axis=2).astype(np.float32)
    r = np.where(deg > 0, 1.0 / np.maximum(deg, 1.0), 0.0).astype(np.float32)
    M = adj.astype(np.float32) * adjw * r[:, :, None]
    MT = np.ascontiguousarray(M.transpose(0, 2, 1).astype(np.float16))
    return [{"hT": hT[b], "W": Wf, "MT": MT[b]} for b in range(B)]


def _run(h, adj, adj_weight, W, a=None, trace=False, **trace_kw):
    nc = _get_nc()
    res = run_bass_kernel_spmd(
        nc, _in_maps(h, adj, adj_weight, W), core_ids=list(range(B)),
        trace=trace, **trace_kw,
    )
    out = np.stack([res.results[c]["out"] for c in range(B)], axis=0)
    return out.astype(np.float32), res


def kernel(h, adj, adj_weight, W, a=None, **_ignored):
    # The NTFF trace path needs an axon hook module this container lacks;
    # make sure an ambient BASS_TRACE can't divert the graded run into it.
    os.environ["BASS_NEVER_TRACE"] = "1"
    out, _ = _run(h, adj, adj_weight, W)
    return out


# revision 15
# speedup vs baseline: 2.8971x; 1.4173x over previous
"""GAT kernel for Trainium2, SPMD over 8 NeuronCores.

Math: the reference GAT variant computes attention logits e[b,h,i,j] that do
NOT depend on j (the "untransposed Wh2" formulation), so softmax over a row
whose support (adj!=0) carries a constant value collapses to 1/deg(i) on the
support and 0 elsewhere (NEG_INF -> exp underflow -> exactly 0 in fp32).
Hence, per batch element b:

    out[b] = elu( (diag(1/deg_b) * adj_b * adj_weight_b) @ (h_b @ W) )

The result is head-independent and `a` is unused.  Sharding: data-parallel
over batch (B == n_cores == 8).

Host-side prep (off the device critical path): transposes, fp16 casts, and
M'^T = (diag(1/deg) @ (adj * adj_weight))^T -- the 1/deg row scale is folded
into M' so the device does no scaling at all.  Device work per core is only
MM1 = h@W (64 matmuls), MM2 = M'@Wh (36 matmuls), and the ELU.

Device schedule (single pass):
  - all input DMAs back-to-back on the Sync sequencer's HWDGE ring, ordered
    so MM1's first chunks land earliest (h-d0, W-d0, h-d1, W-d1, then
    d-pairs, M'^T last);
  - junk warmup matmuls keep the PE p-state ramp alive while the first
    chunks stream (PE ramps 0.65 -> 1.2 -> 2.4 GHz over ~3us of continuous
    execution);
  - MM1 computes the f0 column half first (DMA-arrival-gated, one matmul
    group per chunk), with the f1 half emitted lagging two chunk-pairs
    behind: the f1 backlog keeps the PE saturated over arrival jitter and
    covers the Wh-evacuation latency, and f0 finishing ~5us early widens
    the window for the scalar/vector post-processing pipeline;
  - Wh evacuation PSUM->SBUF split scalar/vector; MM2 reuses the PSUM banks
    (pool rotation inserts the WAR dependency on the evacuation reads);
  - ELU uses elu(x) = min(relu(x), exp(x)-1) as a 3-engine pipeline at MM2
    block cadence: Exp on scalar (PSUM->f16), exp-1 on gpsimd (SBUF f16),
    and one fused scalar_tensor_tensor on vector computing
    min(max(psum, 0), exp-1) straight from PSUM.  The last block is split
    2x256 to shorten the drain chain.  Output is fp16 (cast on host).
"""

import os

import numpy as np

import concourse.bass as bass
import concourse.tile as tile
from concourse import bacc, mybir
from concourse.bass import ts
from concourse.bass_utils import run_bass_kernel_spmd

B, N, D = 8, 512, 1024
P = 128  # SBUF partitions
NB = N // P  # 4 row blocks
DB = D // P  # 8 contraction blocks

F32 = mybir.dt.float32
F16 = mybir.dt.float16
AF = mybir.ActivationFunctionType
ALU = mybir.AluOpType

N_WARM = 6


def build_nc():
    nc = bacc.Bacc("TRN2", target_bir_lowering=False, debug=False, num_devices=B)

    hT = nc.dram_tensor("hT", [D, N], F16, kind="ExternalInput").ap()
    W = nc.dram_tensor("W", [D, D], F16, kind="ExternalInput").ap()
    MT = nc.dram_tensor("MT", [N, N], F16, kind="ExternalInput").ap()
    out = nc.dram_tensor("out", [N, D], F16, kind="ExternalOutput").ap()

    hT_r = hT.rearrange("(d p) i -> p d i", p=P)   # [128, 8, 512]
    W_r = W.rearrange("(d p) f -> p d f", p=P)     # [128, 8, 1024]
    MT_r = MT.rearrange("(j p) i -> p j i", p=P)   # [128, 4, 512]
    out_r = out.rearrange("(i p) f -> p i f", p=P)  # [128, 4, 1024]

    with tile.TileContext(nc) as tc:
        with (
            tc.tile_pool(name="singles", bufs=1) as singles,
            tc.tile_pool(name="work", bufs=3) as work,
            tc.tile_pool(name="outp", bufs=3) as outp,
            tc.tile_pool(name="psum", bufs=8, space="PSUM") as psum,
        ):
            hT_sb = singles.tile([P, DB, N], F16)
            W_sb = singles.tile([P, DB, D], F16)
            MT_sb = singles.tile([P, NB, N], F16)
            Wh = [singles.tile([P, NB, 512], F16, name=f"Wh{f}") for f in range(2)]
            junk = singles.tile([P, 512], F16)

            # ---- input DMAs, one ring, arrival-ordered for MM1 ---------
            # d-pair chunks keep 2-4KB partition lines for DMA efficiency.
            for dp in range(DB // 2):
                nc.sync.dma_start(hT_sb[:, ts(dp, 2)], hT_r[:, ts(dp, 2)])
                nc.sync.dma_start(W_sb[:, ts(dp, 2)], W_r[:, ts(dp, 2)])
            nc.sync.dma_start(MT_sb, MT_r)

            # ---- PE warmup on junk: keeps the p-state ramp alive while
            # the first real chunks stream.  Targets a real MM1 bank
            # (overwritten with start=True later), so no extra PSUM use.
            # PSUM: one tile per 512-wide block so dependency tracking stays
            # per-bank (a shared multi-bank tile would serialize MM2 blocks
            # against the previous block's ELU reads).
            ps1 = [
                [psum.tile([P, 512], F32, name=f"ps1_{f}_{i}", tag="bank") for i in range(NB)]
                for f in range(2)
            ]
            nc.gpsimd.memset(junk, 0.0)
            for _ in range(N_WARM):
                nc.tensor.matmul(
                    ps1[0][0], junk[:, :P], junk, start=True, stop=True
                )

            # ---- MM1: Wh = h @ W; f0 half first, f1 lags two pairs -----
            def mm1(d, f):
                for i in range(NB):
                    nc.tensor.matmul(
                        ps1[f][i],
                        hT_sb[:, d, ts(i, P)],
                        W_sb[:, d, ts(f, 512)],
                        start=(d == 0),
                        stop=(d == DB - 1),
                    )

            def evac(f):
                for i in range(NB):
                    eng = nc.scalar.copy if i % 2 == 0 else nc.vector.tensor_copy
                    eng(Wh[f][:, i], ps1[f][i])

            for dp in range(DB // 2):
                mm1(2 * dp, 0)
                mm1(2 * dp + 1, 0)
                if 1 <= dp < DB // 2 - 1:
                    mm1(2 * (dp - 1), 1)
                    mm1(2 * (dp - 1) + 1, 1)

            # evacuate Wh f0 while the PE chews the f1 backlog
            evac(0)
            for d in range(DB - 4, DB):
                mm1(d, 1)
            evac(1)

            # ---- MM2 + ELU pipeline ------------------------------------
            # elu(x) = min(relu(x), exp(x) - 1), exact for all x
            def mm2(i, f):
                ps2 = psum.tile([P, 512], F32, name=f"ps2_{f}_{i}", tag="bank")
                for j in range(NB):
                    nc.tensor.matmul(
                        ps2,
                        MT_sb[:, j, ts(i, P)],
                        Wh[f][:, j],
                        start=(j == 0),
                        stop=(j == NB - 1),
                    )
                exp_t = work.tile([P, 512], F16, tag="exp")
                em1_t = work.tile([P, 512], F16, tag="em1")
                o_t = outp.tile([P, 512], F16)
                nc.scalar.activation(exp_t, ps2, AF.Exp)
                nc.vector.tensor_scalar_add(em1_t, exp_t, -1.0)
                nc.vector.scalar_tensor_tensor(
                    o_t, ps2, 0.0, em1_t, op0=ALU.max, op1=ALU.min
                )
                nc.sync.dma_start(out_r[:, i, ts(f, 512)], o_t)

            for f in range(2):
                for i in range(NB):
                    mm2(i, f)

    nc.compile()
    return nc


_NC = None


def _get_nc():
    global _NC
    if _NC is None:
        _NC = build_nc()
    return _NC


def _in_maps(h, adj, adj_weight, W):
    h = np.asarray(h, dtype=np.float32)
    adj = np.asarray(adj)
    adjw = np.asarray(adj_weight, dtype=np.float32)
    Wf = np.ascontiguousarray(np.asarray(W, dtype=np.float32).reshape(D, D).astype(np.float16))
    hT = np.ascontiguousarray(h.transpose(0, 2, 1).astype(np.float16))
    deg = adj.sum(axis=2).astype(np.float32)
    r = np.where(deg > 0, 1.0 / np.maximum(deg, 1.0), 0.0).astype(np.float32)
    M = adj.astype(np.float32) * adjw * r[:, :, None]
    MT = np.ascontiguousarray(M.transpose(0, 2, 1).astype(np.float16))
    return [{"hT": hT[b], "W": Wf, "MT": MT[b]} for b in range(B)]


def _run(h, adj, adj_weight, W, a=None, trace=False, **trace_kw):
    nc = _get_nc()
    res = run_bass_kernel_spmd(
        nc, _in_maps(h, adj, adj_weight, W), core_ids=list(range(B)),
        trace=trace, **trace_kw,
    )
    out = np.stack([res.results[c]["out"] for c in range(B)], axis=0)
    return out.astype(np.float32), res


def kernel(h, adj, adj_weight, W, a=None, **_ignored):
    # The NTFF trace path needs an axon hook module this container lacks;
    # make sure an ambient BASS_TRACE can't divert the graded run into it.
    os.environ["BASS_NEVER_TRACE"] = "1"
    out, _ = _run(h, adj, adj_weight, W)
    return out
